# revision 10
# baseline (speedup 1.0000x reference)
"""Chamfer distance loss (per-cluster, bidirectional) on 8 Trainium2 cores.

Problem: points [131072, 3] in 128 equal clusters of 1024. Per cluster c:
  d[i,j] = ||a_i - b_j||^2 ; loss_c = sum_i min_j d + sum_j min_i d
Total = sum of loss_c over clusters 0..126 (the max cluster id is excluded).

Strategy (data-parallel over clusters, 16 clusters/core):
  - Host packs, per cluster, two K=5 operand matrices:
        A_op = [ax; ay; az; 1; aa]   (5 x 1024)
        B_op = [-2bx; -2by; -2bz; bb; 1]
    so that A_op^T @ B_op == d directly (PE emits the distance matrix).
  - Pass 1: lhsT = A_op chunk (i on PSUM partitions), rhs = B_op -> d[i,j].
  - Pass 2: lhsT = B_op chunk, rhs = A_op -> d^T (min over i becomes a row min).
  - Row mins via one DVE tensor_tensor_reduce per [128,1024] block:
    accum_out = min over both j-halves (op0=min folds halves, op1=min reduces).
  - Device outputs raw block mins [128 lanes, 256 cols]; host sums and masks.
"""

import numpy as np

C = 128          # clusters
P = 1024         # points per cluster
DIM = 3
K = 5            # augmented contraction dim
N_CORES = 8
CPC = C // N_CORES   # clusters per core (16)
ICH = P // 128       # i-chunks per cluster (8)
OUT_COLS = 2 * CPC * ICH  # 256

_cache = {}


def _build():
    import concourse.bacc as bacc
    import concourse.mybir as mybir
    from concourse.tile import TileContext

    nc = bacc.Bacc(
        "TRN2", target_bir_lowering=False, debug=False, num_devices=N_CORES)
    f32 = mybir.dt.float32

    a_d = nc.dram_tensor("a_op", [K, CPC * P], f32, kind="ExternalInput")
    b_d = nc.dram_tensor("b_op", [K, CPC * P], f32, kind="ExternalInput")
    out_d = nc.dram_tensor("out", [128, OUT_COLS], f32, kind="ExternalOutput")

    with TileContext(nc) as tc:
        with (
            tc.tile_pool(name="const", bufs=1) as cpool,
            tc.tile_pool(name="psum", bufs=4, space="PSUM") as ppool,
            tc.tile_pool(name="scratch", bufs=4) as spool,
        ):
            a_t = cpool.tile([K, CPC * P], f32)
            b_t = cpool.tile([K, CPC * P], f32)
            # cluster-aligned chunks so each matmul depends on one DMA
            nq = 2
            w = CPC * P // nq
            for q in range(nq):
                nc.sync.dma_start(
                    out=a_t[:, q * w:(q + 1) * w], in_=a_d[:, q * w:(q + 1) * w])
                nc.sync.dma_start(
                    out=b_t[:, q * w:(q + 1) * w], in_=b_d[:, q * w:(q + 1) * w])
            mins = cpool.tile([128, OUT_COLS], f32)

            for dirn in range(2):
                s_t, m_t = (a_t, b_t) if dirn == 0 else (b_t, a_t)
                for c in range(CPC):
                    cs = c * P
                    for ic in range(ICH):
                        ps0 = ppool.tile([128, 512], f32, tag="ps0")
                        ps1 = ppool.tile([128, 512], f32, tag="ps1")
                        lhsT = s_t[:, cs + ic * 128:cs + (ic + 1) * 128]
                        nc.tensor.matmul(
                            ps0[:], lhsT, m_t[:, cs:cs + 512],
                            start=True, stop=True)
                        nc.tensor.matmul(
                            ps1[:], lhsT, m_t[:, cs + 512:cs + P],
                            start=True, stop=True)
                        # DVE can read only one PSUM operand; ScalarE stages
                        # the second j-half into SBUF. (tensor_tensor_reduce
                        # would fuse the next two DVE ops but crashes TRN2.)
                        j1c = spool.tile([128, 512], f32, tag="j1copy")
                        nc.any.tensor_copy(j1c[:], ps1[:])
                        m0 = spool.tile([128, 512], f32, tag="m0")
                        nc.vector.tensor_tensor(
                            out=m0[:], in0=ps0[:], in1=j1c[:],
                            op=mybir.AluOpType.min)
                        col = (dirn * CPC + c) * ICH + ic
                        nc.vector.tensor_reduce(
                            out=mins[:, col:col + 1], in_=m0[:],
                            axis=mybir.AxisListType.X, op=mybir.AluOpType.min)

            nc.sync.dma_start(out=out_d[:], in_=mins[:])
    nc.compile()
    return nc


def _prep(input_points, output_points):
    a = np.ascontiguousarray(input_points, dtype=np.float32).reshape(C, P, DIM)
    b = np.ascontiguousarray(output_points, dtype=np.float32).reshape(C, P, DIM)
    aa = np.einsum("cpd,cpd->cp", a, a).astype(np.float32)
    bb = np.einsum("cpd,cpd->cp", b, b).astype(np.float32)

    a_op = np.empty((C, K, P), np.float32)
    a_op[:, 0:3] = a.transpose(0, 2, 1)
    a_op[:, 3] = 1.0
    a_op[:, 4] = aa

    b_op = np.empty((C, K, P), np.float32)
    b_op[:, 0:3] = -2.0 * b.transpose(0, 2, 1)
    b_op[:, 3] = bb
    b_op[:, 4] = 1.0

    in_maps = []
    for i in range(N_CORES):
        sl = slice(i * CPC, (i + 1) * CPC)
        # [CPC, K, P] -> [K, CPC*P]
        in_maps.append({
            "a_op": np.ascontiguousarray(
                a_op[sl].transpose(1, 0, 2).reshape(K, CPC * P)),
            "b_op": np.ascontiguousarray(
                b_op[sl].transpose(1, 0, 2).reshape(K, CPC * P)),
        })
    return in_maps


def run(inputs, trace=False, trace_kwargs=None):
    """Returns (loss ndarray shape (), BassKernelResults)."""
    from concourse.bass_utils import run_bass_kernel_spmd

    if "nc" not in _cache:
        _cache["nc"] = _build()
    nc = _cache["nc"]

    in_maps = _prep(inputs["input_points"], inputs["output_points"])
    res = run_bass_kernel_spmd(
        nc, in_maps, list(range(N_CORES)),
        trace=trace, **(trace_kwargs or {}))

    # out[core]: [128, 256]; col = (dir*CPC + c)*ICH + ic; sum all lanes/chunks
    per_cluster = np.concatenate([
        res.results[i]["out"].reshape(128, 2, CPC, ICH).sum(
            axis=(0, 1, 3), dtype=np.float64)
        for i in range(N_CORES)
    ])  # [C]

    nb = int(np.max(inputs["input_clusters"]))
    mask = np.arange(C) < nb
    total = np.float32(per_cluster[mask].sum())
    return np.array(total, dtype=np.float32), res


def kernel(input_points, input_clusters, output_points, output_clusters):
    loss, _ = run({
        "input_points": input_points,
        "input_clusters": input_clusters,
        "output_points": output_points,
        "output_clusters": output_clusters,
    })
    return loss


# revision 11
# speedup vs baseline: 1.0003x; 1.0003x over previous
"""Chamfer distance loss (per-cluster, bidirectional) on 8 Trainium2 cores.

Problem: points [131072, 3] in 128 equal clusters of 1024. Per cluster c:
  d[i,j] = ||a_i - b_j||^2 ; loss_c = sum_i min_j d + sum_j min_i d
Total = sum of loss_c over clusters 0..126 (the max cluster id is excluded).

Strategy (data-parallel over clusters, 16 clusters/core):
  - Host packs, per cluster, two K=5 operand matrices:
        A_op = [ax; ay; az; 1; aa]   (5 x 1024)
        B_op = [-2bx; -2by; -2bz; bb; 1]
    so that A_op^T @ B_op == d directly (PE emits the distance matrix).
  - Pass 1: lhsT = A_op chunk (i on PSUM partitions), rhs = B_op -> d[i,j].
  - Pass 2: lhsT = B_op chunk, rhs = A_op -> d^T (min over i becomes a row min).
  - Row mins via one DVE tensor_tensor_reduce per [128,1024] block:
    accum_out = min over both j-halves (op0=min folds halves, op1=min reduces).
  - Device outputs raw block mins [128 lanes, 256 cols]; host sums and masks.
"""

import numpy as np

C = 128          # clusters
P = 1024         # points per cluster
DIM = 3
K = 5            # augmented contraction dim
N_CORES = 8
CPC = C // N_CORES   # clusters per core (16)
ICH = P // 128       # i-chunks per cluster (8)
OUT_COLS = 2 * CPC * ICH  # 256

_cache = {}


def _build():
    import concourse.bacc as bacc
    import concourse.mybir as mybir
    from concourse.tile import TileContext

    nc = bacc.Bacc(
        "TRN2", target_bir_lowering=False, debug=False, num_devices=N_CORES)
    f32 = mybir.dt.float32

    a_d = nc.dram_tensor("a_op", [K, CPC * P], f32, kind="ExternalInput")
    b_d = nc.dram_tensor("b_op", [K, CPC * P], f32, kind="ExternalInput")
    out_d = nc.dram_tensor("out", [128, OUT_COLS], f32, kind="ExternalOutput")

    with TileContext(nc) as tc:
        with (
            tc.tile_pool(name="const", bufs=1) as cpool,
            tc.tile_pool(name="psum", bufs=4, space="PSUM") as ppool,
            tc.tile_pool(name="scratch", bufs=4) as spool,
        ):
            a_t = cpool.tile([K, CPC * P], f32)
            b_t = cpool.tile([K, CPC * P], f32)
            # cluster-aligned chunks so each matmul depends on one DMA
            nq = 2
            w = CPC * P // nq
            for q in range(nq):
                nc.sync.dma_start(
                    out=a_t[:, q * w:(q + 1) * w], in_=a_d[:, q * w:(q + 1) * w])
                nc.sync.dma_start(
                    out=b_t[:, q * w:(q + 1) * w], in_=b_d[:, q * w:(q + 1) * w])
            mins = cpool.tile([128, OUT_COLS], f32)

            for dirn in range(2):
                s_t, m_t = (a_t, b_t) if dirn == 0 else (b_t, a_t)
                for c in range(CPC):
                    cs = c * P
                    for ic in range(ICH):
                        ps = ppool.tile([128, P], f32, tag="ps")
                        lhsT = s_t[:, cs + ic * 128:cs + (ic + 1) * 128]
                        nc.tensor.matmul(
                            ps[:, 0:512], lhsT, m_t[:, cs:cs + 512],
                            start=True, stop=True)
                        nc.tensor.matmul(
                            ps[:, 512:1024], lhsT, m_t[:, cs + 512:cs + P],
                            start=True, stop=True)
                        col = (dirn * CPC + c) * ICH + ic
                        # single DVE reduce over the 2-bank PSUM tile
                        # (tensor_tensor_reduce would halve DVE time but
                        # crashes TRN2)
                        nc.vector.tensor_reduce(
                            out=mins[:, col:col + 1], in_=ps[:],
                            axis=mybir.AxisListType.X, op=mybir.AluOpType.min)

            nc.sync.dma_start(out=out_d[:], in_=mins[:])
    nc.compile()
    return nc


def _prep(input_points, output_points):
    a = np.ascontiguousarray(input_points, dtype=np.float32).reshape(C, P, DIM)
    b = np.ascontiguousarray(output_points, dtype=np.float32).reshape(C, P, DIM)
    aa = np.einsum("cpd,cpd->cp", a, a).astype(np.float32)
    bb = np.einsum("cpd,cpd->cp", b, b).astype(np.float32)

    a_op = np.empty((C, K, P), np.float32)
    a_op[:, 0:3] = a.transpose(0, 2, 1)
    a_op[:, 3] = 1.0
    a_op[:, 4] = aa

    b_op = np.empty((C, K, P), np.float32)
    b_op[:, 0:3] = -2.0 * b.transpose(0, 2, 1)
    b_op[:, 3] = bb
    b_op[:, 4] = 1.0

    in_maps = []
    for i in range(N_CORES):
        sl = slice(i * CPC, (i + 1) * CPC)
        # [CPC, K, P] -> [K, CPC*P]
        in_maps.append({
            "a_op": np.ascontiguousarray(
                a_op[sl].transpose(1, 0, 2).reshape(K, CPC * P)),
            "b_op": np.ascontiguousarray(
                b_op[sl].transpose(1, 0, 2).reshape(K, CPC * P)),
        })
    return in_maps


def run(inputs, trace=False, trace_kwargs=None):
    """Returns (loss ndarray shape (), BassKernelResults)."""
    from concourse.bass_utils import run_bass_kernel_spmd

    if "nc" not in _cache:
        _cache["nc"] = _build()
    nc = _cache["nc"]

    in_maps = _prep(inputs["input_points"], inputs["output_points"])
    res = run_bass_kernel_spmd(
        nc, in_maps, list(range(N_CORES)),
        trace=trace, **(trace_kwargs or {}))

    # out[core]: [128, 256]; col = (dir*CPC + c)*ICH + ic; sum all lanes/chunks
    per_cluster = np.concatenate([
        res.results[i]["out"].reshape(128, 2, CPC, ICH).sum(
            axis=(0, 1, 3), dtype=np.float64)
        for i in range(N_CORES)
    ])  # [C]

    nb = int(np.max(inputs["input_clusters"]))
    mask = np.arange(C) < nb
    total = np.float32(per_cluster[mask].sum())
    return np.array(total, dtype=np.float32), res


def kernel(input_points, input_clusters, output_points, output_clusters):
    loss, _ = run({
        "input_points": input_points,
        "input_clusters": input_clusters,
        "output_points": output_points,
        "output_clusters": output_clusters,
    })
    return loss


# revision 13
# speedup vs baseline: 2.9055x; 2.9046x over previous
"""Chamfer distance loss (per-cluster, bidirectional) on 8 Trainium2 cores.

Problem: points [131072, 3] in 128 equal clusters of 1024. Per cluster c:
  d[i,j] = ||a_i - b_j||^2 ; loss_c = sum_i min_j d + sum_j min_i d
Total = sum of loss_c over clusters 0..126 (the max cluster id is excluded).

Strategy (data-parallel over clusters, 16 clusters/core):
  - Host packs, per cluster, two K=5 operand matrices:
        A_op = [ax; ay; az; 1; aa]   (5 x 1024)
        B_op = [-2bx; -2by; -2bz; bb; 1]
    so that A_op^T @ B_op == d directly (PE emits the distance matrix).
  - Pass 1: lhsT = A_op chunk (i on PSUM partitions), rhs = B_op -> d[i,j].
  - Pass 2: lhsT = B_op chunk, rhs = A_op -> d^T (min over i becomes a row min).
  - Row mins via one DVE tensor_tensor_reduce per [128,1024] block:
    accum_out = min over both j-halves (op0=min folds halves, op1=min reduces).
  - Device outputs raw block mins [128 lanes, 256 cols]; host sums and masks.
"""

import numpy as np

C = 128          # clusters
P = 1024         # points per cluster
DIM = 3
K = 13           # augmented contraction dim (split-fp16 rows)
N_CORES = 8
CPC = C // N_CORES   # clusters per core (16)
ICH = P // 128       # i-chunks per cluster (8)
OUT_COLS = 2 * CPC * ICH  # 256

_cache = {}


def _build():
    import concourse.bacc as bacc
    import concourse.mybir as mybir
    from concourse.tile import TileContext

    nc = bacc.Bacc(
        "TRN2", target_bir_lowering=False, debug=False, num_devices=N_CORES)
    f32 = mybir.dt.float32
    f16 = mybir.dt.float16

    a_d = nc.dram_tensor("a_op", [K, CPC * P], f16, kind="ExternalInput")
    b_d = nc.dram_tensor("b_op", [K, CPC * P], f16, kind="ExternalInput")
    out_d = nc.dram_tensor("out", [128, OUT_COLS], f32, kind="ExternalOutput")

    with TileContext(nc) as tc:
        with (
            tc.tile_pool(name="const", bufs=1) as cpool,
            tc.tile_pool(name="psum", bufs=4, space="PSUM") as ppool,
            tc.tile_pool(name="scratch", bufs=4) as spool,
        ):
            a_t = cpool.tile([K, CPC * P], f16)
            b_t = cpool.tile([K, CPC * P], f16)
            # cluster-aligned chunks so each matmul depends on one DMA
            nq = 2
            w = CPC * P // nq
            for q in range(nq):
                nc.sync.dma_start(
                    out=a_t[:, q * w:(q + 1) * w], in_=a_d[:, q * w:(q + 1) * w])
                nc.sync.dma_start(
                    out=b_t[:, q * w:(q + 1) * w], in_=b_d[:, q * w:(q + 1) * w])
            mins = cpool.tile([128, OUT_COLS], f32)

            for dirn in range(2):
                s_t, m_t = (a_t, b_t) if dirn == 0 else (b_t, a_t)
                for c in range(CPC):
                    cs = c * P
                    for ic in range(ICH):
                        ps = ppool.tile([128, P], f32, tag="ps")
                        lhsT = s_t[:, cs + ic * 128:cs + (ic + 1) * 128]
                        nc.tensor.matmul(
                            ps[:, 0:512], lhsT, m_t[:, cs:cs + 512],
                            start=True, stop=True)
                        nc.tensor.matmul(
                            ps[:, 512:1024], lhsT, m_t[:, cs + 512:cs + P],
                            start=True, stop=True)
                        col = (dirn * CPC + c) * ICH + ic
                        # single DVE reduce over the 2-bank PSUM tile
                        # (tensor_tensor_reduce would halve DVE time but
                        # crashes TRN2)
                        nc.vector.tensor_reduce(
                            out=mins[:, col:col + 1], in_=ps[:],
                            axis=mybir.AxisListType.X, op=mybir.AluOpType.min)

            nc.sync.dma_start(out=out_d[:], in_=mins[:])
    nc.compile()
    return nc


def _split(x):
    """fp32 -> (hi, lo) fp16 pair with x ~= hi + lo."""
    hi = x.astype(np.float16)
    lo = (x - hi.astype(np.float32)).astype(np.float16)
    return hi, lo


def _prep(input_points, output_points):
    a = np.ascontiguousarray(input_points, dtype=np.float32).reshape(C, P, DIM)
    b = np.ascontiguousarray(output_points, dtype=np.float32).reshape(C, P, DIM)
    aa = np.einsum("cpd,cpd->cp", a, a).astype(np.float32)
    bb = np.einsum("cpd,cpd->cp", b, b).astype(np.float32)

    at = a.transpose(0, 2, 1)            # [C,3,P]
    bt2 = -2.0 * b.transpose(0, 2, 1)    # [C,3,P]  (B = -2b)
    ah, al = _split(at)
    bh, bl = _split(bt2)
    aah, aal = _split(aa)
    bbh, bbl = _split(bb)

    # d = sum_k A[k,i] * B[k,j]:
    #   ah.Bh + al.Bh + ah.Bl  (= -2ab)   rows 0-2, 3-5, 6-8
    #   1*bbh + 1*bbl                      rows 9, 10
    #   aah*1 + aal*1                      rows 11, 12
    a_op = np.empty((C, K, P), np.float16)
    a_op[:, 0:3] = ah
    a_op[:, 3:6] = al
    a_op[:, 6:9] = ah
    a_op[:, 9:11] = 1.0
    a_op[:, 11] = aah
    a_op[:, 12] = aal

    b_op = np.empty((C, K, P), np.float16)
    b_op[:, 0:3] = bh
    b_op[:, 3:6] = bh
    b_op[:, 6:9] = bl
    b_op[:, 9] = bbh
    b_op[:, 10] = bbl
    b_op[:, 11:13] = 1.0

    in_maps = []
    for i in range(N_CORES):
        sl = slice(i * CPC, (i + 1) * CPC)
        # [CPC, K, P] -> [K, CPC*P]
        in_maps.append({
            "a_op": np.ascontiguousarray(
                a_op[sl].transpose(1, 0, 2).reshape(K, CPC * P)),
            "b_op": np.ascontiguousarray(
                b_op[sl].transpose(1, 0, 2).reshape(K, CPC * P)),
        })
    return in_maps


def run(inputs, trace=False, trace_kwargs=None):
    """Returns (loss ndarray shape (), BassKernelResults)."""
    from concourse.bass_utils import run_bass_kernel_spmd

    if "nc" not in _cache:
        _cache["nc"] = _build()
    nc = _cache["nc"]

    in_maps = _prep(inputs["input_points"], inputs["output_points"])
    res = run_bass_kernel_spmd(
        nc, in_maps, list(range(N_CORES)),
        trace=trace, **(trace_kwargs or {}))

    # out[core]: [128, 256]; col = (dir*CPC + c)*ICH + ic; sum all lanes/chunks
    per_cluster = np.concatenate([
        res.results[i]["out"].reshape(128, 2, CPC, ICH).sum(
            axis=(0, 1, 3), dtype=np.float64)
        for i in range(N_CORES)
    ])  # [C]

    nb = int(np.max(inputs["input_clusters"]))
    mask = np.arange(C) < nb
    total = np.float32(per_cluster[mask].sum())
    return np.array(total, dtype=np.float32), res


def kernel(input_points, input_clusters, output_points, output_clusters):
    loss, _ = run({
        "input_points": input_points,
        "input_clusters": input_clusters,
        "output_points": output_points,
        "output_clusters": output_clusters,
    })
    return loss
